# revision 27
# baseline (speedup 1.0000x reference)
"""Attention-pooling kernel for Trainium2, 8-core data-parallel.

Math (per batch row b):
  att_in[t] = [q, k_t, q - k_t]  (192)
  h = sigmoid(att_in @ W1 + b1)  (32)
  s_t = h @ W2 + b2
  w = softmax(mask ? s : -2^32+1)
  out = sum_t w_t k_t            (64)

Algebraic simplification: att_in @ W1 = q @ Wq + k @ Wkk with
  Wq  = W1[0:64] + W1[128:192]
  Wkk = W1[64:128] - W1[128:192]
b2 is a constant shift -> softmax-invariant, dropped.

Layout strategy per core (512 rows, chunks of 64):
  keys loaded t-major [t, b, d]; per-b PE transposes build keysT [d, (b,t)];
  projection matmul (K=64, col-tiled 4x into a [128,400] psum) + accumulated
  q matmul (broadcast rhs); ACT sigmoid (bias=b1) -> hsigT; W2-pattern scores
  matmul -> [4,400]; SBUF scatter to [b,t]; masked softmax (fused exp+sum);
  w normalized then transposed; weighted sum as w-stationary matmuls with
  diagonal extraction.
"""

import os
import numpy as np

B, T, D, H = 4096, 200, 64, 32
NCORES = 8
BPC = B // NCORES          # 512 rows per core
NB = 64                    # chunk of batch rows
NCHUNK = BPC // NB         # 8
T1, T2 = 128, T - 128      # 128 + 72
NEG_INF = float(np.float32(-(2.0**32) + 1.0))

F32 = None  # set in _build (mybir.dt.float32)
BUILD_VARIANT = "full"


def _build_nc(use_f32r=True):
    from contextlib import ExitStack
    import concourse.bacc as bacc
    import concourse.bass as bass
    import concourse.tile as tile
    import concourse.mybir as mybir
    from concourse.masks import make_identity

    f32 = mybir.dt.float32
    bf16 = mybir.dt.bfloat16
    u8 = mybir.dt.uint8

    nc = bacc.Bacc("TRN2", target_bir_lowering=False,
                   dynamic_dma_scratch_size=24576)

    # DRAM I/O. float32r is bit-identical to float32; np side sees float32.
    q_d = nc.dram_tensor("query", [BPC, 1, D], f32, kind="ExternalInput")
    k_d = nc.dram_tensor("keys", [BPC, T, D], f32, kind="ExternalInput")
    m_d = nc.dram_tensor("mask", [BPC, T], u8, kind="ExternalInput")
    w1_d = nc.dram_tensor("W1", [3 * D, H], f32, kind="ExternalInput")
    b1_d = nc.dram_tensor("b1", [H], f32, kind="ExternalInput")
    w2_d = nc.dram_tensor("W2", [H, 1], f32, kind="ExternalInput")
    out_d = nc.dram_tensor("out", [BPC, 1, D], f32, kind="ExternalOutput")

    AF = mybir.ActivationFunctionType
    ALU = mybir.AluOpType
    AX = mybir.AxisListType

    with ExitStack() as ctx:
        tc = ctx.enter_context(tile.TileContext(nc))
        consts = ctx.enter_context(tc.tile_pool(name="consts", bufs=1))
        kpool = ctx.enter_context(tc.tile_pool(name="kpool", bufs=NCHUNK))
        mkp = ctx.enter_context(tc.tile_pool(name="mkp", bufs=NCHUNK))
        sbt = ctx.enter_context(tc.tile_pool(name="sbt", bufs=NCHUNK))
        ktp = ctx.enter_context(tc.tile_pool(name="ktp", bufs=2))
        hpool = ctx.enter_context(tc.tile_pool(name="hpool", bufs=3))
        spool = ctx.enter_context(tc.tile_pool(name="spool", bufs=2))
        opool = ctx.enter_context(tc.tile_pool(name="opool", bufs=2))
        # PSUM pools: 8 banks total.
        pst = ctx.enter_context(tc.tile_pool(name="pst", bufs=2, space="PSUM"))
        psp = ctx.enter_context(tc.tile_pool(name="psp", bufs=2, space="PSUM"))
        pss = ctx.enter_context(tc.tile_pool(name="pss", bufs=1, space="PSUM"))
        pso = ctx.enter_context(tc.tile_pool(name="pso", bufs=1, space="PSUM"))

        # ---- one-time constants ----
        ident = consts.tile([128, 128], f32)
        make_identity(nc, ident)
        identb = consts.tile([128, 128], bf16)
        nc.vector.tensor_copy(out=identb, in_=ident)

        # W1 slices replicated on both partition halves
        w1abc = consts.tile([128, 3 * H], f32)
        for i in range(3):
            nc.sync.dma_start(out=w1abc[0:D, i * H:(i + 1) * H],
                              in_=w1_d[i * D:(i + 1) * D, :])
            nc.sync.dma_start(out=w1abc[D:2 * D, i * H:(i + 1) * H],
                              in_=w1_d[i * D:(i + 1) * D, :])
        wkkS = consts.tile([128, H], bf16)
        wqS = consts.tile([128, H], bf16)
        nc.vector.tensor_tensor(out=wkkS, in0=w1abc[:, H:2 * H],
                                in1=w1abc[:, 2 * H:3 * H], op=ALU.subtract)
        nc.vector.tensor_tensor(out=wqS, in0=w1abc[:, 0:H],
                                in1=w1abc[:, 2 * H:3 * H], op=ALU.add)
        # block-diagonal [128, 64]: rows 0:64 -> cols 0:32, rows 64:128 -> 32:64
        wkk2 = consts.tile([128, 2 * H], bf16)
        wq2 = consts.tile([128, 2 * H], bf16)
        nc.vector.memset(wkk2, 0.0)
        nc.vector.memset(wq2, 0.0)
        nc.vector.tensor_copy(out=wkk2[0:D, 0:H], in_=wkkS[0:D, :])
        nc.vector.tensor_copy(out=wkk2[D:128, H:2 * H], in_=wkkS[D:128, :])
        nc.vector.tensor_copy(out=wq2[0:D, 0:H], in_=wqS[0:D, :])
        nc.vector.tensor_copy(out=wq2[D:128, H:2 * H], in_=wqS[D:128, :])

        b1x4 = consts.tile([128, 1], f32)
        for j in range(4):
            nc.sync.dma_start(out=b1x4[j * H:(j + 1) * H, :], in_=b1_d[:, None])

        w2x4 = consts.tile([128, 4], bf16)
        nc.vector.memset(w2x4, 0.0)
        for j in range(4):
            nc.gpsimd.dma_start(out=w2x4[j * H:(j + 1) * H, j:j + 1], in_=w2_d[:, :])

        neginf = consts.tile([NB, T], f32)
        nc.vector.memset(neginf, NEG_INF)

        # transposed output accumulator [d, b] for the whole core
        outT_sb = consts.tile([D, BPC], f32)

        TP = T // 2  # 100 t-pairs; partition tp holds t = 2*tp, 2*tp+1

        def dma_load(ci):
            b0 = ci * NB
            # kb [tp, b, (e,d)] loaded bf16 directly: SWDGE casts f32->bf16
            # in the DMA datapath (512B contiguous source runs).
            kb = kpool.tile([TP, NB, 2 * D], bf16, tag="kb")
            for h in range(4):
                bh = b0 + h * (NB // 4)
                nc.gpsimd.dma_start(
                    out=kb[:, h * (NB // 4):(h + 1) * (NB // 4), :],
                    in_=k_d[bh:bh + NB // 4, :, :].rearrange(
                        "b (tp e) d -> tp b (e d)", e=2))
            qin = hpool.tile([NB, D], f32, tag="qin")
            nc.sync.dma_start(out=qin, in_=q_d[b0:b0 + NB, 0, :])
            mk = mkp.tile([NB, T], u8, tag="mk")
            nc.sync.dma_start(out=mk, in_=m_d[b0:b0 + NB, :])
            return dict(kb=kb, qin=qin, mk=mk)

        def compute(ci, tl):
            b0 = ci * NB
            kb, qin, mk = tl["kb"], tl["qin"], tl["mk"]

            # ---- query transpose; qT2 = qT replicated on both halves ----
            ps_q = pst.tile([D, NB], f32, tag="pst")
            nc.tensor.transpose(ps_q, qin, ident[0:NB, 0:NB])
            qT = hpool.tile([D, NB], bf16, tag="qT")
            nc.vector.tensor_copy(out=qT, in_=ps_q)
            qT2 = hpool.tile([128, NB], bf16, tag="qT2")
            nc.sync.dma_start(out=qT2[0:D, :], in_=qT)
            nc.sync.dma_start(out=qT2[D:128, :], in_=qT)

            # ---- transposes -> keysT2 [(e,d), b, tp] ----
            # col order per quad: [b0, b0+2, b0+1, b0+3] so j-pairs are adjacent
            # 8 transposes share one PSUM tile -> 1 wide copy (fewer ACT/DVE
            # fixed costs).
            keysT2 = ktp.tile([128, NB, TP], bf16)
            for q8 in range(NB // 8):
                psA = pst.tile([128, 8, TP], bf16, tag="pst")
                for p8 in range(8):
                    q2 = 4 * q8 + p8 // 2
                    g, half, p = q2 // 2, q2 % 2, p8 % 2
                    b = 4 * g + half + 2 * p
                    nc.tensor.transpose(psA[:, p8, :], kb[:, b, :],
                                        identb[0:TP, 0:TP])
                if q8 % 4 == 0:
                    nc.scalar.copy(out=keysT2[:, 8 * q8:8 * q8 + 8, :],
                                   in_=psA)
                else:
                    nc.vector.tensor_copy(out=keysT2[:, 8 * q8:8 * q8 + 8, :],
                                          in_=psA)

            if BUILD_VARIANT == "transp_only":
                return
            # ---- projection + sigmoid + scores (2 groups = 8 b per pass) ----
            sc_sb = spool.tile([4, NB // 4, 2, TP], f32, tag="scsb")
            for g in range(0, NB // 4, 2):
                ps_pre = psp.tile([128, 2, 2, TP], f32, tag="psp")
                for gg in range(2):
                    for j in range(2):
                        c0 = 4 * (g + gg) + 2 * j
                        nc.tensor.matmul(
                            ps_pre[64 * j:64 * (j + 1), gg, :, :], lhsT=wkk2,
                            rhs=keysT2[:, c0:c0 + 2, :],
                            start=True, stop=False, tile_position=(0, 64 * j))
                        qslice = qT2[:, 4 * (g + gg) + j:4 * (g + gg) + j + 1]
                        rhs_q = bass.AP(
                            tensor=qslice.tensor, offset=qslice.offset,
                            ap=[qslice.ap[0], [2, 2], [0, TP]])
                        nc.tensor.matmul(
                            ps_pre[64 * j:64 * (j + 1), gg, :, :], lhsT=wq2,
                            rhs=rhs_q,
                            start=False, stop=True, tile_position=(0, 64 * j))
                hsigT = hpool.tile([128, 2, 2, TP], bf16, tag="hsig")
                nc.scalar.activation(out=hsigT, in_=ps_pre, func=AF.Sigmoid,
                                     bias=b1x4[:, 0:1])
                ps_sc = pss.tile([4, 2, 2, TP], f32, tag="pss")
                nc.tensor.matmul(ps_sc, lhsT=w2x4, rhs=hsigT,
                                 start=True, stop=True)
                if g % 8 == 0:
                    nc.scalar.copy(out=sc_sb[:, g:g + 2, :, :], in_=ps_sc)
                else:
                    nc.vector.tensor_copy(out=sc_sb[:, g:g + 2, :, :],
                                          in_=ps_sc)

            return dict(kb=kb, mk=mk, sc_sb=sc_sb)

        def scatter(ci, st):
            sc_sb = st["sc_sb"]
            # ---- scatter scores: s_bt2[4g+2c2+j, e, tp] = sc_sb[2j+e, g, c2, tp]
            s_bt2 = sbt.tile([NB, 2, TP], f32, tag="sbt")
            for c in range(4):
                j, e = c // 2, c % 2
                nc.sync.dma_start(out=s_bt2[j:NB:2, e, :],
                                  in_=sc_sb[c:c + 1, :, :, :])
            st["s_bt2"] = s_bt2

        def finish(ci, st):
            b0 = ci * NB
            kb, mk, s_bt2 = st["kb"], st["mk"], st["s_bt2"]

            # mask restrided to [b, (e, tp)]
            mke = spool.tile([NB, 2, TP], u8, tag="mke")
            mkv = bass.AP(tensor=mk.tensor, offset=mk.offset,
                          ap=[mk.ap[0], [1, 2], [2, TP]])
            nc.vector.tensor_copy(out=mke, in_=mkv)

            # ---- masked softmax over t (parity-split free layout) ----
            sm = spool.tile([NB, 2, TP], f32, tag="sm")
            nc.vector.tensor_copy(out=sm, in_=neginf[:, 0:T])
            nc.vector.copy_predicated(out=sm, mask=mke, data=s_bt2)
            nmx = spool.tile([NB, 1], f32, tag="nmx")
            nc.vector.tensor_reduce(out=nmx, in_=sm, axis=AX.XY, op=ALU.max,
                                    negate=True)
            wexp = spool.tile([NB, 2, TP], f32, tag="wexp")
            ssum = spool.tile([NB, 1], f32, tag="ssum")
            nc.scalar.activation(out=wexp, in_=sm, func=AF.Exp,
                                 bias=nmx[:, 0:1], accum_out=ssum[:, 0:1])
            rs = spool.tile([NB, 1], f32, tag="rs")
            nc.vector.reciprocal(out=rs, in_=ssum)
            wn = spool.tile([NB, 2, TP], bf16, tag="wn")
            nc.vector.tensor_scalar(out=wn, in0=wexp, scalar1=rs[:, 0:1],
                                    scalar2=None, op0=ALU.mult)

            # ---- transpose w by parity -> wTe/wTo [tp, b] (bf16 path) ----
            ps_w1 = pst.tile([TP, NB], bf16, tag="pst")
            nc.tensor.transpose(ps_w1, wn[:, 0, :], identb[0:NB, 0:NB])
            wTe = spool.tile([TP, NB], bf16, tag="wTe")
            nc.vector.tensor_copy(out=wTe, in_=ps_w1)
            ps_w2 = pst.tile([TP, NB], bf16, tag="pst")
            nc.tensor.transpose(ps_w2, wn[:, 1, :], identb[0:NB, 0:NB])
            wTo = spool.tile([TP, NB], bf16, tag="wTo")
            nc.vector.tensor_copy(out=wTo, in_=ps_w2)

            if BUILD_VARIANT == "no_wsum":
                return
            # keys-stationary weighted sum: per b, two accumulating matmuls
            # (even/odd t) with w as the 1-column moving operand ->
            # outT [d, b] in PSUM, one copy per chunk into the accumulator.
            ps_oT = pso.tile([D, NB], f32, tag="pso")
            for b in range(NB):
                nc.tensor.matmul(ps_oT[:, b:b + 1], lhsT=kb[:, b, 0:D],
                                 rhs=wTe[:, b:b + 1],
                                 start=True, stop=False)
                nc.tensor.matmul(ps_oT[:, b:b + 1], lhsT=kb[:, b, D:2 * D],
                                 rhs=wTo[:, b:b + 1],
                                 start=False, stop=True)
            nc.vector.tensor_copy(out=outT_sb[:, b0:b0 + NB], in_=ps_oT)

        sts = []
        for ci in range(NCHUNK):
            st = compute(ci, dma_load(ci))
            if st is not None:
                scatter(ci, st)
                sts.append(st)
        if sts:
            # order all Sigmoids before all Exps on ACT so the compiler
            # emits 3 act-table loads instead of 2 per chunk
            nc.scalar.drain()
        for ci, st in enumerate(sts):
            finish(ci, st)

        # ---- end phase: transpose outT [d, b] -> [b, d] and store ----
        if BUILD_VARIANT == "full":
            for t4 in range(BPC // 128):
                ps_f = pst.tile([128, D], f32, tag="pstf")
                nc.tensor.transpose(ps_f, outT_sb[:, 128 * t4:128 * (t4 + 1)],
                                    ident[0:D, 0:D])
                ob = opool.tile([128, D], f32, tag="ob")
                nc.vector.tensor_copy(out=ob, in_=ps_f)
                nc.sync.dma_start(out=out_d[128 * t4:128 * (t4 + 1), 0, :],
                                  in_=ob)

    nc.compile()
    return nc


_NC_CACHE = {}
_LAST_RESULT = None


def _get_nc(use_f32r=True):
    if use_f32r not in _NC_CACHE:
        _NC_CACHE[use_f32r] = _build_nc(use_f32r)
    return _NC_CACHE[use_f32r]


def kernel(query, keys, mask, W1, b1, W2, b2, _trace=False):
    from concourse.bass_utils import run_bass_kernel_spmd

    query = np.ascontiguousarray(np.asarray(query, dtype=np.float32))
    keys = np.ascontiguousarray(np.asarray(keys, dtype=np.float32))
    mask_u8 = np.ascontiguousarray(np.asarray(mask)).astype(np.uint8)
    W1 = np.ascontiguousarray(np.asarray(W1, dtype=np.float32))
    b1f = np.ascontiguousarray(np.asarray(b1, dtype=np.float32))
    W2 = np.ascontiguousarray(np.asarray(W2, dtype=np.float32))

    nc = _get_nc(use_f32r=os.environ.get("KERNEL_F32R", "1") == "1")

    in_maps = []
    for c in range(NCORES):
        lo, hi = c * BPC, (c + 1) * BPC
        in_maps.append({
            "query": query[lo:hi],
            "keys": keys[lo:hi],
            "mask": mask_u8[lo:hi],
            "W1": W1,
            "b1": b1f,
            "W2": W2,
        })

    res = run_bass_kernel_spmd(nc, in_maps, core_ids=list(range(NCORES)),
                               trace=_trace)
    global _LAST_RESULT
    _LAST_RESULT = res
    out = np.concatenate([r["out"] for r in res.results], axis=0)
    return out.astype(np.float32)


if __name__ == "__main__":
    rng = np.random.default_rng(0)
    q = rng.standard_normal((B, 1, D), dtype=np.float32)
    k = rng.standard_normal((B, T, D), dtype=np.float32)
    m = rng.integers(0, 2, size=(B, T)) > 0
    m[:, 0] = True
    W1 = rng.standard_normal((3 * D, H), dtype=np.float32) * 0.1
    b1 = np.zeros(H, np.float32)
    W2 = rng.standard_normal((H, 1), dtype=np.float32) * 0.25
    b2 = np.zeros(1, np.float32)
    o = kernel(query=q, keys=k, mask=m, W1=W1, b1=b1, W2=W2, b2=b2)
    print("out", o.shape, o.dtype, float(np.abs(o).max()))

